# revision 13
# baseline (speedup 1.0000x reference)
"""Bahdanau-attention kernel for 8 Trainium2 NeuronCores (SPMD data-parallel).

Math (per batch b):
    proj_f = features @ W1 + b1          [L, U]
    proj_h = hidden @ W2 + b2            [U]
    score  = tanh(proj_f + proj_h)       [L, U]
    logits = score @ V (+ bV, dropped: softmax shift-invariant)   [L]
    attn   = softmax(logits over L)      [L]
    ctx    = sum_l attn[l] * features[l, :]                        [D]

Sharding: batch 64 -> 8 cores x 8 batches. Weights replicated.

All matmuls run in float32r (TF32, 1 cycle/row at N>=512 — measured same
cadence as bf16, ~4e-4 accuracy) except tiny fp32 helper matmuls.

Per core/batch pipeline (PE kept dense, in-order):
  - warmup matmul stream flips the HAM clock gate to 2.4 GHz while the
    startup DMAs (hidT, W2aug, W1, fT[0]) are in flight.
  - bh^T = (hidden@W2 + b1 + b2)^T via hidT-stationary matmuls (out [8,512])
    + PE transposes with an 8x8 identity.
  - proj phase: proj_f^T = W1-block (stationary) x fT (moving) -> PSUM
    [128u, 512l]; ACT tanh (+per-partition bias) -> f32r score; V-column
    matmuls into logits PSUM [1,1024], delayed one group so the PE never
    waits on ACT.
  - softmax (DVE reduce_max negated, ACT Exp with accumulated sum, DVE
    reciprocal) right after the proj phase — no PE involvement.
  - post phase (emitted after the NEXT batch's proj phase, so its PE work
    slots into the stream): attn-weight row -> [128,8] via K=1 column
    matmuls; ctx accumulation over natural-layout features; outputs scaled
    by 1/sum during PSUM->SBUF copies.
"""

import sys

import numpy as np

sys.path.insert(0, "/opt/trn_rl_repo")

import ml_dtypes  # noqa: E402

import concourse.bass as bass  # noqa: E402
from concourse import bacc, mybir  # noqa: E402
from concourse.bass_utils import run_bass_kernel_spmd  # noqa: E402
from concourse.masks import make_identity  # noqa: E402
from concourse.tile import TileContext  # noqa: E402

B, L, D, H, UNITS = 64, 1024, 1024, 1024, 1024
NCORES = 8
BPC = B // NCORES  # batches per core
P = 128
KD = D // P  # 8 k-blocks over D
KH = 9  # k-blocks over H+1 padded to 9*128
NU = UNITS // P  # 8 u-blocks
NL = L // P  # 8 l-blocks
N_WARM = 40

F32 = mybir.dt.float32
F32R = mybir.dt.float32r
BF16 = mybir.dt.bfloat16
Act = mybir.ActivationFunctionType

_PROGRAM_CACHE = {}


def build_program():
    nc = bacc.Bacc()

    fT = nc.declare_dram_parameter("fT", [BPC, 2, P, KD, 512], F32R, isOutput=False)
    fN = nc.declare_dram_parameter("fN", [BPC, P, NL, D], F32R, isOutput=False)
    W1d = nc.declare_dram_parameter("W1", [2, P, KD, 512], F32R, isOutput=False)
    W2d = nc.declare_dram_parameter("W2aug", [3, P, 3, UNITS], F32R, isOutput=False)
    hidTd = nc.declare_dram_parameter("hidT", [P, KH, BPC], F32R, isOutput=False)
    Vd = nc.declare_dram_parameter("Vp", [P, NU], F32R, isOutput=False)
    ctx_out = nc.declare_dram_parameter("ctx", [BPC, D], F32, isOutput=True)
    attn_out = nc.declare_dram_parameter("attn", [BPC, L], F32, isOutput=True)

    with TileContext(nc) as tc:
        with (
            tc.tile_pool(name="const", bufs=1) as const,
            tc.tile_pool(name="ftp", bufs=4) as ftp,
            tc.tile_pool(name="fnp", bufs=2) as fnp,
            tc.tile_pool(name="scp", bufs=4) as scp,
            tc.tile_pool(name="small", bufs=2) as small,
            tc.tile_pool(name="outp", bufs=1) as outp,
            tc.tile_pool(name="pproj", bufs=3, space="PSUM") as pproj,
            tc.tile_pool(name="plog", bufs=1, space="PSUM") as plog,
            tc.tile_pool(name="pwt2", bufs=1, space="PSUM") as pwt2,
            tc.tile_pool(name="pctx", bufs=2, space="PSUM") as pctx,
        ):
            # ---- startup: warmup stream interleaved with bh compute ----
            # DMA order: hidT, W2 chunks (for bh), W1 first half, V; the
            # first batch's fT halves follow inside proj_phase(0).
            warm_sb = const.tile([P, 512], BF16)
            nc.gpsimd.memset(warm_sb[:], 0.0)
            hidT_sb = const.tile([P, KH, BPC], F32R)
            nc.sync.dma_start(hidT_sb[:], hidTd[:])
            w2c = [
                fnp.tile([P, NL, D], F32R, tag="fnb", name=f"w2c{c}")
                for c in range(3)
            ]
            for c in range(2):
                nc.sync.dma_start(w2c[c][:, 0:3, :], W2d[c])
            w1h = [const.tile([P, KD, 512], F32R, name=f"w1h{h}") for h in range(2)]
            nc.sync.dma_start(w1h[0][:], W1d[0])
            nc.sync.dma_start(w2c[2][:, 0:3, :], W2d[2])
            v_sb = const.tile([P, NU], F32R)
            nc.sync.dma_start(v_sb[:], Vd[:])
            w1b_started = [False]
            ident8 = const.tile([8, 8], F32)
            make_identity(nc, ident8[:])
            ones_sb = const.tile([1, 1], F32)
            nc.vector.memset(ones_sb[:], 1.0)

            warm_ps = pproj.tile([P, 512], F32, tag="proj")

            def warm(n):
                for _ in range(n):
                    nc.tensor.matmul(
                        warm_ps[:], warm_sb[:, :P], warm_sb[:], start=True, stop=True
                    )

            # bh = hidden @ W2aug, accumulated over chunked W2; out [8, 1024]
            bh_sb = const.tile([8, UNITS], F32)
            bh_ps = [
                pctx.tile([8, 512], F32, tag="ctx", name=f"bh_ps{uh}") for uh in range(2)
            ]
            warm(18)
            for c in range(3):
                for k in range(3):
                    for uh in range(2):
                        nc.tensor.matmul(
                            bh_ps[uh][:],
                            hidT_sb[:, 3 * c + k, :],
                            w2c[c][:, k, uh * 512 : (uh + 1) * 512],
                            start=(c == 0 and k == 0),
                            stop=(c == 2 and k == 2),
                        )
                if c < 2:
                    warm(6)
            for uh in range(2):
                nc.vector.tensor_copy(bh_sb[:, uh * 512 : (uh + 1) * 512], bh_ps[uh][:])
            bhT_ps = pwt2.tile([P, NU * BPC], F32, tag="wt2", name="bhT_ps")
            for ub in range(NU):
                nc.tensor.matmul(
                    bhT_ps[:, ub * BPC : (ub + 1) * BPC],
                    bh_sb[:, ub * P : (ub + 1) * P],
                    ident8[:],
                    is_transpose=True,
                    start=(ub == 0),
                    stop=(ub == NU - 1),
                )
            bhT_sb = const.tile([P, NU * BPC], F32)
            nc.vector.tensor_copy(bhT_sb[:], bhT_ps[:])
            warm(10)

            soft = {}

            def proj_phase(b):
                fTh = [
                    ftp.tile([P, KD, 512], F32R, tag="fTh", name=f"fTh{h}")
                    for h in range(2)
                ]
                nc.sync.dma_start(fTh[0][:], fT[b, 0])
                if not w1b_started[0]:
                    nc.sync.dma_start(w1h[1][:], W1d[1])
                    w1b_started[0] = True
                nc.sync.dma_start(fTh[1][:], fT[b, 1])
                fnb = fnp.tile([P, NL, D], F32R, tag="fnb", name="fnb")
                nc.sync.dma_start(fnb[:], fN[b])

                logits_ps = plog.tile([1, L], F32, name="logits_ps")
                pending_v = None

                def flush_v(final=False):
                    nonlocal pending_v
                    if pending_v is None:
                        return
                    pub, plh, pscore = pending_v
                    nc.tensor.matmul(
                        logits_ps[:, plh * 512 : (plh + 1) * 512],
                        v_sb[:, pub : pub + 1],
                        pscore[:],
                        start=(pub == 0),
                        stop=(pub == NU - 1),
                    )
                    pending_v = None

                for lh in range(2):
                    for ub in range(NU):
                        ps = pproj.tile([P, 512], F32, tag="proj", name="ps")
                        for k in range(KD):
                            nc.tensor.matmul(
                                ps[:],
                                w1h[ub // 4][:, k, (ub % 4) * P : (ub % 4 + 1) * P],
                                fTh[lh][:, k, :],
                                start=(k == 0),
                                stop=(k == KD - 1),
                            )
                        flush_v()
                        score = scp.tile([P, 512], F32R, name="score")
                        nc.scalar.activation(
                            score[:],
                            ps[:],
                            Act.Tanh,
                            bias=bhT_sb[:, ub * BPC + b : ub * BPC + b + 1],
                        )
                        pending_v = (ub, lh, score)
                flush_v(final=True)

                # softmax over L on partition 0 (ACT/DVE only, no PE)
                negmax = small.tile([1, 1], F32, tag="negmax", name="negmax")
                nc.vector.tensor_reduce(
                    negmax[:],
                    logits_ps[:],
                    axis=mybir.AxisListType.X,
                    op=mybir.AluOpType.max,
                    negate=True,
                )
                e_sb = small.tile([1, L], F32, tag="e", name="e_sb")
                esum = small.tile([1, 1], F32, tag="esum", name="esum")
                nc.scalar.activation(
                    e_sb[:],
                    logits_ps[:],
                    Act.Exp,
                    bias=negmax[:],
                    accum_out=esum[:],
                )
                rsum = small.tile([1, 1], F32, tag="rsum", name="rsum")
                nc.vector.reciprocal(rsum[:], esum[:])

                if b < BPC - 1:
                    e8 = small.tile([8, P], F32, tag="e8", name="e8")
                    for t in range(8):
                        nc.gpsimd.dma_start(
                            e8[t : t + 1, :], e_sb[:, t * P : (t + 1) * P]
                        )
                else:
                    e8 = None

                soft[b] = (e_sb, e8, rsum, fnb)

            def post_phase(b):
                e_sb, e8, rsum, fnb = soft.pop(b)

                # wT [128, NL]: transpose of scattered e8 (or direct column
                # matmuls for the last batch, whose scatter would be exposed)
                wt_ps = pwt2.tile([P, NU * BPC], F32, tag="wt2", name="wt_ps")
                if e8 is not None:
                    nc.tensor.matmul(
                        wt_ps[:, :NL],
                        e8[:],
                        ident8[:],
                        is_transpose=True,
                        start=True,
                        stop=True,
                    )
                else:
                    for lt in range(NL):
                        nc.tensor.matmul(
                            wt_ps[:, lt : lt + 1],
                            e_sb[:, lt * P : (lt + 1) * P],
                            ones_sb[:],
                            start=(lt == 0),
                            stop=(lt == NL - 1),
                        )
                wt_sb = small.tile([P, NL], F32R, tag="wtsb", name="wt_sb")
                nc.vector.tensor_copy(wt_sb[:], wt_ps[:, :NL])

                # attention weights output
                aw_sb = outp.tile([1, L], F32, tag="aw", name="aw_sb")
                nc.scalar.activation(aw_sb[:], e_sb[:], Act.Copy, scale=rsum[:])
                nc.sync.dma_start(attn_out[b : b + 1, :], aw_sb[:])

                # context: ctx[d] = sum_l w[l] * fN[l, d]
                ctx_sb = outp.tile([1, D], F32, tag="ctxsb", name="ctx_sb")
                for dh in range(2):
                    ctx_ps = pctx.tile([1, 512], F32, tag="ctx", name="ctx_ps")
                    for lt in range(NL):
                        nc.tensor.matmul(
                            ctx_ps[:],
                            wt_sb[:, lt : lt + 1],
                            fnb[:, lt, dh * 512 : (dh + 1) * 512],
                            start=(lt == 0),
                            stop=(lt == NL - 1),
                        )
                    nc.scalar.activation(
                        ctx_sb[:, dh * 512 : (dh + 1) * 512],
                        ctx_ps[:],
                        Act.Copy,
                        scale=rsum[:],
                    )
                nc.sync.dma_start(ctx_out[b : b + 1, :], ctx_sb[:])

            for b in range(BPC):
                proj_phase(b)
                if b > 0:
                    post_phase(b - 1)
            post_phase(BPC - 1)

    nc.finalize()
    return nc


def _get_program():
    if "nc" not in _PROGRAM_CACHE:
        _PROGRAM_CACHE["nc"] = build_program()
    return _PROGRAM_CACHE["nc"]


def _prep_in_maps(features, hidden, W1, b1, W2, b2, V, bV):
    features = np.asarray(features, dtype=np.float32)
    hidden = np.asarray(hidden, dtype=np.float32)
    W1 = np.asarray(W1, dtype=np.float32)
    b1 = np.asarray(b1, dtype=np.float32)
    W2 = np.asarray(W2, dtype=np.float32)
    b2 = np.asarray(b2, dtype=np.float32)
    V = np.asarray(V, dtype=np.float32)

    # W2aug: rows of W2, then a row of (b1 + b2), zero-padded to KH*128 rows,
    # pre-arranged to [chunk c, partition p, k-in-chunk, u].
    W2aug = np.zeros((KH * P, UNITS), dtype=np.float32)
    W2aug[:H] = W2
    W2aug[H] = b1 + b2
    W2aug = np.ascontiguousarray(
        W2aug.reshape(3, 3, P, UNITS).transpose(0, 2, 1, 3)
    )
    W1p = np.ascontiguousarray(
        W1.reshape(KD, P, 2, 512).transpose(2, 1, 0, 3)
    )
    Vp = np.ascontiguousarray(V.reshape(NU, P).T)

    in_maps = []
    for c in range(NCORES):
        sl = slice(c * BPC, (c + 1) * BPC)
        feats = features[sl]
        fT = np.ascontiguousarray(
            feats.transpose(0, 2, 1)
            .reshape(BPC, KD, P, 2, 512)
            .transpose(0, 3, 2, 1, 4)
        )
        fN = np.ascontiguousarray(
            feats.reshape(BPC, NL, P, D).transpose(0, 2, 1, 3)
        )
        hidT = np.zeros((KH * P, BPC), dtype=np.float32)
        hidT[:H] = hidden[sl].T
        hidT[H] = 1.0
        hidT = np.ascontiguousarray(hidT.reshape(KH, P, BPC).transpose(1, 0, 2))
        in_maps.append(
            {
                "fT": fT,
                "fN": fN,
                "W1": W1p,
                "W2aug": W2aug,
                "hidT": hidT,
                "Vp": Vp,
            }
        )
    return in_maps


def kernel(features, hidden, W1, b1, W2, b2, V, bV, _trace=False):
    nc = _get_program()
    in_maps = _prep_in_maps(features, hidden, W1, b1, W2, b2, V, bV)
    res = run_bass_kernel_spmd(
        nc, in_maps, core_ids=list(range(NCORES)), trace=_trace
    )
    ctx = np.concatenate([res.results[c]["ctx"] for c in range(NCORES)], axis=0)
    attn = np.concatenate([res.results[c]["attn"] for c in range(NCORES)], axis=0)
    out = (
        np.ascontiguousarray(ctx, dtype=np.float32),
        np.ascontiguousarray(attn.reshape(B, L, 1), dtype=np.float32),
    )
    if _trace:
        return out, res
    return out


# revision 14
# speedup vs baseline: 1.0000x; 1.0000x over previous
"""Bahdanau-attention kernel for 8 Trainium2 NeuronCores (SPMD data-parallel).

Math (per batch b):
    proj_f = features @ W1 + b1          [L, U]
    proj_h = hidden @ W2 + b2            [U]
    score  = tanh(proj_f + proj_h)       [L, U]
    logits = score @ V (+ bV, dropped: softmax shift-invariant)   [L]
    attn   = softmax(logits over L)      [L]
    ctx    = sum_l attn[l] * features[l, :]                        [D]

Sharding: batch 64 -> 8 cores x 8 batches. Weights replicated.

All matmuls run in float32r (TF32, 1 cycle/row at N>=512 — measured same
cadence as bf16, ~4e-4 accuracy) except tiny fp32 helper matmuls.

All inputs are pre-arranged on the host into their exact on-chip layouts so
every DMA is a per-partition-contiguous block transfer.

Per core/batch pipeline (PE kept dense, in-order):
  - warmup matmul stream flips the HAM clock gate to 2.4 GHz while the
    startup DMAs stream, interleaved with the bh^T = (hidden@W2 + b1 + b2)^T
    computation (hidT-stationary matmuls + PE transpose via 8x8 identity).
  - proj phase: proj_f^T = W1-block (stationary) x fT (moving) in float32r
    -> PSUM [128u, 512l]; ACT tanh (+per-partition bias column of bh^T) ->
    f32r score; V-column matmuls into logits PSUM [1,1024], emission delayed
    one group so the in-order PE never waits on ACT.
  - softmax (DVE reduce_max negated, ACT Exp with accumulated sum, DVE
    reciprocal) right after the proj phase — no PE involvement; the e-row is
    scattered to [8,128] by tiny gpsimd DMAs (except the last batch).
  - post phase (emitted after the NEXT batch's proj phase so its PE work
    slots into the stream): attn-weight column tile [128,8] via one PE
    transpose (direct K=1 column matmuls for the last batch, whose scatter
    latency would be exposed); ctx accumulated over natural-layout features;
    outputs scaled by 1/sum during the PSUM->SBUF copies.
"""

import sys

import numpy as np

sys.path.insert(0, "/opt/trn_rl_repo")

from concourse import bacc, mybir  # noqa: E402
from concourse.bass_utils import run_bass_kernel_spmd  # noqa: E402
from concourse.masks import make_identity  # noqa: E402
from concourse.tile import TileContext  # noqa: E402

B, L, D, H, UNITS = 64, 1024, 1024, 1024, 1024
NCORES = 8
BPC = B // NCORES  # batches per core
P = 128
KD = D // P  # 8 k-blocks over D
KH = 9  # k-blocks over H+1 padded to 9*128
NU = UNITS // P  # 8 u-blocks
NL = L // P  # 8 l-blocks

F32 = mybir.dt.float32
F32R = mybir.dt.float32r
BF16 = mybir.dt.bfloat16
Act = mybir.ActivationFunctionType

_PROGRAM_CACHE = {}


def build_program():
    nc = bacc.Bacc()

    fT = nc.declare_dram_parameter("fT", [BPC, 2, P, KD, 512], F32R, isOutput=False)
    fN = nc.declare_dram_parameter("fN", [BPC, P, NL, D], F32R, isOutput=False)
    W1d = nc.declare_dram_parameter("W1", [2, P, KD, 512], F32R, isOutput=False)
    W2d = nc.declare_dram_parameter("W2aug", [3, P, 3, UNITS], F32R, isOutput=False)
    hidTd = nc.declare_dram_parameter("hidT", [P, KH, BPC], F32R, isOutput=False)
    Vd = nc.declare_dram_parameter("Vp", [P, NU], F32R, isOutput=False)
    ctx_out = nc.declare_dram_parameter("ctx", [BPC, D], F32, isOutput=True)
    attn_out = nc.declare_dram_parameter("attn", [BPC, L], F32, isOutput=True)

    with TileContext(nc) as tc:
        with (
            tc.tile_pool(name="const", bufs=1) as const,
            tc.tile_pool(name="ftp", bufs=4) as ftp,
            tc.tile_pool(name="fnp", bufs=2) as fnp,
            tc.tile_pool(name="scp", bufs=4) as scp,
            tc.tile_pool(name="small", bufs=2) as small,
            tc.tile_pool(name="outp", bufs=1) as outp,
            tc.tile_pool(name="pproj", bufs=3, space="PSUM") as pproj,
            tc.tile_pool(name="plog", bufs=1, space="PSUM") as plog,
            tc.tile_pool(name="pwt2", bufs=1, space="PSUM") as pwt2,
            tc.tile_pool(name="pctx", bufs=2, space="PSUM") as pctx,
        ):
            # ---- startup: warmup stream interleaved with bh compute ----
            # DMA order: hidT, W2 chunks (for bh), W1 first half, V; the
            # first batch's fT halves follow inside proj_phase(0).
            warm_sb = const.tile([P, 512], BF16)
            nc.gpsimd.memset(warm_sb[:], 0.0)
            hidT_sb = const.tile([P, KH, BPC], F32R)
            nc.sync.dma_start(hidT_sb[:], hidTd[:])
            w2c = [
                fnp.tile([P, NL, D], F32R, tag="fnb", name=f"w2c{c}")
                for c in range(3)
            ]
            for c in range(2):
                nc.sync.dma_start(w2c[c][:, 0:3, :], W2d[c])
            w1h = [const.tile([P, KD, 512], F32R, name=f"w1h{h}") for h in range(2)]
            nc.sync.dma_start(w1h[0][:], W1d[0])
            nc.sync.dma_start(w2c[2][:, 0:3, :], W2d[2])
            v_sb = const.tile([P, NU], F32R)
            nc.sync.dma_start(v_sb[:], Vd[:])
            w1b_started = [False]
            ident8 = const.tile([8, 8], F32)
            make_identity(nc, ident8[:])
            ones_sb = const.tile([1, 1], F32)
            nc.vector.memset(ones_sb[:], 1.0)

            warm_ps = pproj.tile([P, 512], F32, tag="proj")

            def warm(n):
                for _ in range(n):
                    nc.tensor.matmul(
                        warm_ps[:], warm_sb[:, :P], warm_sb[:], start=True, stop=True
                    )

            # bh = hidden @ W2aug, accumulated over chunked W2; out [8, 1024]
            bh_sb = const.tile([8, UNITS], F32)
            bh_ps = [
                pctx.tile([8, 512], F32, tag="ctx", name=f"bh_ps{uh}") for uh in range(2)
            ]
            warm(18)
            for c in range(3):
                for k in range(3):
                    for uh in range(2):
                        nc.tensor.matmul(
                            bh_ps[uh][:],
                            hidT_sb[:, 3 * c + k, :],
                            w2c[c][:, k, uh * 512 : (uh + 1) * 512],
                            start=(c == 0 and k == 0),
                            stop=(c == 2 and k == 2),
                        )
                if c < 2:
                    warm(6)
            for uh in range(2):
                nc.vector.tensor_copy(bh_sb[:, uh * 512 : (uh + 1) * 512], bh_ps[uh][:])
            bhT_ps = pwt2.tile([P, NU * BPC], F32, tag="wt2", name="bhT_ps")
            for ub in range(NU):
                nc.tensor.matmul(
                    bhT_ps[:, ub * BPC : (ub + 1) * BPC],
                    bh_sb[:, ub * P : (ub + 1) * P],
                    ident8[:],
                    is_transpose=True,
                    start=(ub == 0),
                    stop=(ub == NU - 1),
                )
            bhT_sb = const.tile([P, NU * BPC], F32)
            nc.vector.tensor_copy(bhT_sb[:], bhT_ps[:])
            warm(10)

            soft = {}

            def proj_phase(b):
                fTh = [
                    ftp.tile([P, KD, 512], F32R, tag="fTh", name=f"fTh{h}")
                    for h in range(2)
                ]
                nc.sync.dma_start(fTh[0][:], fT[b, 0])
                if not w1b_started[0]:
                    nc.sync.dma_start(w1h[1][:], W1d[1])
                    w1b_started[0] = True
                nc.sync.dma_start(fTh[1][:], fT[b, 1])
                fnb = fnp.tile([P, NL, D], F32R, tag="fnb", name="fnb")
                nc.sync.dma_start(fnb[:], fN[b])

                logits_ps = plog.tile([1, L], F32, name="logits_ps")
                pending_v = None

                def flush_v(final=False):
                    nonlocal pending_v
                    if pending_v is None:
                        return
                    pub, plh, pscore = pending_v
                    nc.tensor.matmul(
                        logits_ps[:, plh * 512 : (plh + 1) * 512],
                        v_sb[:, pub : pub + 1],
                        pscore[:],
                        start=(pub == 0),
                        stop=(pub == NU - 1),
                    )
                    pending_v = None

                for lh in range(2):
                    for ub in range(NU):
                        ps = pproj.tile([P, 512], F32, tag="proj", name="ps")
                        for k in range(KD):
                            nc.tensor.matmul(
                                ps[:],
                                w1h[ub // 4][:, k, (ub % 4) * P : (ub % 4 + 1) * P],
                                fTh[lh][:, k, :],
                                start=(k == 0),
                                stop=(k == KD - 1),
                            )
                        flush_v()
                        score = scp.tile([P, 512], F32R, name="score")
                        nc.scalar.activation(
                            score[:],
                            ps[:],
                            Act.Tanh,
                            bias=bhT_sb[:, ub * BPC + b : ub * BPC + b + 1],
                        )
                        pending_v = (ub, lh, score)
                flush_v(final=True)

                # softmax over L on partition 0 (ACT/DVE only, no PE)
                negmax = small.tile([1, 1], F32, tag="negmax", name="negmax")
                nc.vector.tensor_reduce(
                    negmax[:],
                    logits_ps[:],
                    axis=mybir.AxisListType.X,
                    op=mybir.AluOpType.max,
                    negate=True,
                )
                e_sb = small.tile([1, L], F32, tag="e", name="e_sb")
                esum = small.tile([1, 1], F32, tag="esum", name="esum")
                nc.scalar.activation(
                    e_sb[:],
                    logits_ps[:],
                    Act.Exp,
                    bias=negmax[:],
                    accum_out=esum[:],
                )
                rsum = small.tile([1, 1], F32, tag="rsum", name="rsum")
                nc.vector.reciprocal(rsum[:], esum[:])

                if b < BPC - 1:
                    e8 = small.tile([8, P], F32, tag="e8", name="e8")
                    for t in range(8):
                        nc.gpsimd.dma_start(
                            e8[t : t + 1, :], e_sb[:, t * P : (t + 1) * P]
                        )
                else:
                    e8 = None

                soft[b] = (e_sb, e8, rsum, fnb)

            def post_phase(b):
                e_sb, e8, rsum, fnb = soft.pop(b)

                # wT [128, NL]: transpose of scattered e8 (or direct column
                # matmuls for the last batch, whose scatter would be exposed)
                wt_ps = pwt2.tile([P, NU * BPC], F32, tag="wt2", name="wt_ps")
                if e8 is not None:
                    nc.tensor.matmul(
                        wt_ps[:, :NL],
                        e8[:],
                        ident8[:],
                        is_transpose=True,
                        start=True,
                        stop=True,
                    )
                else:
                    for lt in range(NL):
                        nc.tensor.matmul(
                            wt_ps[:, lt : lt + 1],
                            e_sb[:, lt * P : (lt + 1) * P],
                            ones_sb[:],
                            start=(lt == 0),
                            stop=(lt == NL - 1),
                        )
                wt_sb = small.tile([P, NL], F32R, tag="wtsb", name="wt_sb")
                nc.vector.tensor_copy(wt_sb[:], wt_ps[:, :NL])

                # attention weights output
                aw_sb = outp.tile([1, L], F32, tag="aw", name="aw_sb")
                nc.scalar.activation(aw_sb[:], e_sb[:], Act.Copy, scale=rsum[:])
                nc.sync.dma_start(attn_out[b : b + 1, :], aw_sb[:])

                # context: ctx[d] = sum_l w[l] * fN[l, d]
                ctx_sb = outp.tile([1, D], F32, tag="ctxsb", name="ctx_sb")
                for dh in range(2):
                    ctx_ps = pctx.tile([1, 512], F32, tag="ctx", name="ctx_ps")
                    for lt in range(NL):
                        nc.tensor.matmul(
                            ctx_ps[:],
                            wt_sb[:, lt : lt + 1],
                            fnb[:, lt, dh * 512 : (dh + 1) * 512],
                            start=(lt == 0),
                            stop=(lt == NL - 1),
                        )
                    nc.scalar.activation(
                        ctx_sb[:, dh * 512 : (dh + 1) * 512],
                        ctx_ps[:],
                        Act.Copy,
                        scale=rsum[:],
                    )
                nc.sync.dma_start(ctx_out[b : b + 1, :], ctx_sb[:])

            for b in range(BPC):
                proj_phase(b)
                if b > 0:
                    post_phase(b - 1)
            post_phase(BPC - 1)

    nc.finalize()
    return nc


def _get_program():
    if "nc" not in _PROGRAM_CACHE:
        _PROGRAM_CACHE["nc"] = build_program()
    return _PROGRAM_CACHE["nc"]


def _prep_in_maps(features, hidden, W1, b1, W2, b2, V, bV):
    features = np.asarray(features, dtype=np.float32)
    hidden = np.asarray(hidden, dtype=np.float32)
    W1 = np.asarray(W1, dtype=np.float32)
    b1 = np.asarray(b1, dtype=np.float32)
    W2 = np.asarray(W2, dtype=np.float32)
    b2 = np.asarray(b2, dtype=np.float32)
    V = np.asarray(V, dtype=np.float32)

    # W2aug: rows of W2, then a row of (b1 + b2), zero-padded to KH*128 rows,
    # pre-arranged to [chunk c, partition p, k-in-chunk, u].
    W2aug = np.zeros((KH * P, UNITS), dtype=np.float32)
    W2aug[:H] = W2
    W2aug[H] = b1 + b2
    W2aug = np.ascontiguousarray(
        W2aug.reshape(3, 3, P, UNITS).transpose(0, 2, 1, 3)
    )
    W1p = np.ascontiguousarray(
        W1.reshape(KD, P, 2, 512).transpose(2, 1, 0, 3)
    )
    Vp = np.ascontiguousarray(V.reshape(NU, P).T)

    in_maps = []
    for c in range(NCORES):
        sl = slice(c * BPC, (c + 1) * BPC)
        feats = features[sl]
        fT = np.ascontiguousarray(
            feats.transpose(0, 2, 1)
            .reshape(BPC, KD, P, 2, 512)
            .transpose(0, 3, 2, 1, 4)
        )
        fN = np.ascontiguousarray(
            feats.reshape(BPC, NL, P, D).transpose(0, 2, 1, 3)
        )
        hidT = np.zeros((KH * P, BPC), dtype=np.float32)
        hidT[:H] = hidden[sl].T
        hidT[H] = 1.0
        hidT = np.ascontiguousarray(hidT.reshape(KH, P, BPC).transpose(1, 0, 2))
        in_maps.append(
            {
                "fT": fT,
                "fN": fN,
                "W1": W1p,
                "W2aug": W2aug,
                "hidT": hidT,
                "Vp": Vp,
            }
        )
    return in_maps


def kernel(features, hidden, W1, b1, W2, b2, V, bV, _trace=False):
    nc = _get_program()
    in_maps = _prep_in_maps(features, hidden, W1, b1, W2, b2, V, bV)
    res = run_bass_kernel_spmd(
        nc, in_maps, core_ids=list(range(NCORES)), trace=_trace
    )
    ctx = np.concatenate([res.results[c]["ctx"] for c in range(NCORES)], axis=0)
    attn = np.concatenate([res.results[c]["attn"] for c in range(NCORES)], axis=0)
    out = (
        np.ascontiguousarray(ctx, dtype=np.float32),
        np.ascontiguousarray(attn.reshape(B, L, 1), dtype=np.float32),
    )
    if _trace:
        return out, res
    return out


# revision 19
# speedup vs baseline: 1.0686x; 1.0686x over previous
"""Bahdanau-attention kernel for 8 Trainium2 NeuronCores (SPMD data-parallel).

Math (per batch b):
    proj_f = features @ W1 + b1          [L, U]
    proj_h = hidden @ W2 + b2            [U]
    score  = tanh(proj_f + proj_h)       [L, U]
    logits = score @ V (+ bV, dropped: softmax shift-invariant)   [L]
    attn   = softmax(logits over L)      [L]
    ctx    = sum_l attn[l] * features[l, :]                        [D]

Sharding: batch 64 -> 8 cores x 8 batches. Weights replicated.

All matmuls run in float32r (TF32, 1 cycle/row at N>=512 — measured same
cadence as bf16, ~4e-4 accuracy) except tiny fp32 helper matmuls.

All inputs are pre-arranged on the host into their exact on-chip layouts so
every DMA is a per-partition-contiguous block transfer.

Per core/batch pipeline (PE kept dense, in-order):
  - warmup matmul stream flips the HAM clock gate to 2.4 GHz while the
    startup DMAs stream, interleaved with the bh^T = (hidden@W2 + b1 + b2)^T
    computation (hidT-stationary matmuls + PE transpose via 8x8 identity).
  - proj phase: proj_f^T = W1-block (stationary) x fT (moving) in float32r
    -> PSUM [128u, 512l]; ACT tanh (+per-partition bias column of bh^T) ->
    f32r score; V-column matmuls into logits PSUM [1,1024], emission delayed
    one group so the in-order PE never waits on ACT.
  - softmax (DVE reduce_max negated, ACT Exp with accumulated sum, DVE
    reciprocal) right after the proj phase — no PE involvement; the e-row is
    scattered to [8,128] by tiny gpsimd DMAs (except the last batch).
  - post phase (emitted after the NEXT batch's proj phase so its PE work
    slots into the stream): attn-weight column tile [128,8] via one PE
    transpose (direct K=1 column matmuls for the last batch, whose scatter
    latency would be exposed); ctx accumulated over natural-layout features;
    outputs scaled by 1/sum during the PSUM->SBUF copies.
"""

import sys

import numpy as np

sys.path.insert(0, "/opt/trn_rl_repo")

from concourse import bacc, mybir  # noqa: E402
from concourse.bass_utils import run_bass_kernel_spmd  # noqa: E402
from concourse.masks import make_identity  # noqa: E402
from concourse.tile import TileContext  # noqa: E402

B, L, D, H, UNITS = 64, 1024, 1024, 1024, 1024
NCORES = 8
BPC = B // NCORES  # batches per core
P = 128
KD = D // P  # 8 k-blocks over D
KH = 9  # k-blocks over H+1 padded to 9*128
NU = UNITS // P  # 8 u-blocks
NL = L // P  # 8 l-blocks

F32 = mybir.dt.float32
F32R = mybir.dt.float32r
BF16 = mybir.dt.bfloat16
Act = mybir.ActivationFunctionType

_PROGRAM_CACHE = {}


def build_program():
    nc = bacc.Bacc()

    fT = nc.declare_dram_parameter("fT", [BPC, 2, P, KD, 512], F32R, isOutput=False)
    fN = nc.declare_dram_parameter("fN", [BPC, P, NL, D], F32R, isOutput=False)
    W1d = nc.declare_dram_parameter("W1", [2, P, KD, 512], F32R, isOutput=False)
    W2d = nc.declare_dram_parameter("W2aug", [3, P, 3, UNITS], F32R, isOutput=False)
    hidTd = nc.declare_dram_parameter("hidT", [P, KH, BPC], F32R, isOutput=False)
    Vd = nc.declare_dram_parameter("Vp", [P, NU], F32R, isOutput=False)
    ctx_out = nc.declare_dram_parameter("ctx", [BPC, D], F32, isOutput=True)
    attn_out = nc.declare_dram_parameter("attn", [BPC, L], F32, isOutput=True)

    with TileContext(nc) as tc:
        with (
            tc.tile_pool(name="const", bufs=1) as const,
            tc.tile_pool(name="ftp", bufs=4) as ftp,
            tc.tile_pool(name="fnp", bufs=2) as fnp,
            tc.tile_pool(name="scp", bufs=6) as scp,
            tc.tile_pool(name="small", bufs=2) as small,
            tc.tile_pool(name="outp", bufs=1) as outp,
            tc.tile_pool(name="pproj", bufs=3, space="PSUM") as pproj,
            tc.tile_pool(name="plog", bufs=1, space="PSUM") as plog,
            tc.tile_pool(name="pwt2", bufs=1, space="PSUM") as pwt2,
            tc.tile_pool(name="pctx", bufs=2, space="PSUM") as pctx,
        ):
            # ---- startup: warmup stream interleaved with bh compute ----
            # DMA order: hidT, W2 chunks (for bh), W1 first half, V; the
            # first batch's fT halves follow inside proj_phase(0).
            warm_sb = const.tile([P, 512], BF16)
            nc.gpsimd.memset(warm_sb[:], 0.0)
            hidT_sb = const.tile([P, KH, BPC], F32R)
            nc.sync.dma_start(hidT_sb[:], hidTd[:])
            w2c = [
                fnp.tile([P, NL, D], F32R, tag="fnb", name=f"w2c{c}")
                for c in range(3)
            ]
            for c in range(2):
                nc.sync.dma_start(w2c[c][:, 0:3, :], W2d[c])
            w1h = [const.tile([P, KD, 512], F32R, name=f"w1h{h}") for h in range(2)]
            nc.sync.dma_start(w1h[0][:], W1d[0])
            nc.sync.dma_start(w2c[2][:, 0:3, :], W2d[2])
            v_sb = const.tile([P, NU], F32R)
            nc.sync.dma_start(v_sb[:], Vd[:])
            w1b_started = [False]
            ident8 = const.tile([8, 8], F32)
            make_identity(nc, ident8[:])
            ones_sb = const.tile([1, 1], F32)
            nc.vector.memset(ones_sb[:], 1.0)

            warm_ps = pproj.tile([P, 512], F32, tag="proj")

            def warm(n):
                for _ in range(n):
                    nc.tensor.matmul(
                        warm_ps[:], warm_sb[:, :P], warm_sb[:], start=True, stop=True
                    )

            # bh = hidden @ W2aug, accumulated over chunked W2; out [8, 1024]
            bh_sb = const.tile([8, UNITS], F32)
            bh_ps = [
                pctx.tile([8, 512], F32, tag="ctx", name=f"bh_ps{uh}") for uh in range(2)
            ]
            warm(28)
            for c in range(3):
                for k in range(3):
                    for uh in range(2):
                        nc.tensor.matmul(
                            bh_ps[uh][:],
                            hidT_sb[:, 3 * c + k, :],
                            w2c[c][:, k, uh * 512 : (uh + 1) * 512],
                            start=(c == 0 and k == 0),
                            stop=(c == 2 and k == 2),
                        )
                if c < 2:
                    warm(10)
            for uh in range(2):
                nc.vector.tensor_copy(bh_sb[:, uh * 512 : (uh + 1) * 512], bh_ps[uh][:])
            bhT_ps = pwt2.tile([P, NU * BPC], F32, tag="wt2", name="bhT_ps")
            for ub in range(NU):
                nc.tensor.matmul(
                    bhT_ps[:, ub * BPC : (ub + 1) * BPC],
                    bh_sb[:, ub * P : (ub + 1) * P],
                    ident8[:],
                    is_transpose=True,
                    start=(ub == 0),
                    stop=(ub == NU - 1),
                )
            bhT_sb = const.tile([P, NU * BPC], F32)
            nc.vector.tensor_copy(bhT_sb[:], bhT_ps[:])
            warm(12)

            soft = {}

            def proj_phase(b):
                fTh = [
                    ftp.tile([P, KD, 512], F32R, tag="fTh", name=f"fTh{h}")
                    for h in range(2)
                ]
                nc.sync.dma_start(fTh[0][:], fT[b, 0])
                if not w1b_started[0]:
                    nc.sync.dma_start(w1h[1][:], W1d[1])
                    w1b_started[0] = True
                nc.sync.dma_start(fTh[1][:], fT[b, 1])
                fnb = fnp.tile([P, NL, D], F32R, tag="fnb", name="fnb")
                nc.sync.dma_start(fnb[:], fN[b])

                logits_ps = plog.tile([1, L], F32, name="logits_ps")
                pending_v = []
                v_lag = 4 if b == 0 else 2

                def flush_v(keep):
                    while len(pending_v) > keep:
                        pub, plh, pscore = pending_v.pop(0)
                        nc.tensor.matmul(
                            logits_ps[:, plh * 512 : (plh + 1) * 512],
                            v_sb[:, pub : pub + 1],
                            pscore[:],
                            start=(pub == 0),
                            stop=(pub == NU - 1),
                        )

                for lh in range(2):
                    for ub in range(NU):
                        ps = pproj.tile([P, 512], F32, tag="proj", name="ps")
                        for k in range(KD):
                            nc.tensor.matmul(
                                ps[:],
                                w1h[ub // 4][:, k, (ub % 4) * P : (ub % 4 + 1) * P],
                                fTh[lh][:, k, :],
                                start=(k == 0),
                                stop=(k == KD - 1),
                            )
                        flush_v(v_lag)
                        score = scp.tile([P, 512], F32R, name="score")
                        nc.scalar.activation(
                            score[:],
                            ps[:],
                            Act.Tanh,
                            bias=bhT_sb[:, ub * BPC + b : ub * BPC + b + 1],
                        )
                        pending_v.append((ub, lh, score))
                flush_v(0)

                # softmax over L on partition 0 (ACT/DVE only, no PE)
                negmax = small.tile([1, 1], F32, tag="negmax", name="negmax")
                nc.vector.tensor_reduce(
                    negmax[:],
                    logits_ps[:],
                    axis=mybir.AxisListType.X,
                    op=mybir.AluOpType.max,
                    negate=True,
                )
                e_sb = small.tile([1, L], F32, tag="e", name="e_sb")
                esum = small.tile([1, 1], F32, tag="esum", name="esum")
                nc.scalar.activation(
                    e_sb[:],
                    logits_ps[:],
                    Act.Exp,
                    bias=negmax[:],
                    accum_out=esum[:],
                )
                rsum = small.tile([1, 1], F32, tag="rsum", name="rsum")
                nc.vector.reciprocal(rsum[:], esum[:])

                if b < BPC - 1:
                    e8 = small.tile([8, P], F32, tag="e8", name="e8")
                    for t in range(8):
                        nc.gpsimd.dma_start(
                            e8[t : t + 1, :], e_sb[:, t * P : (t + 1) * P]
                        )
                else:
                    e8 = None

                soft[b] = (e_sb, e8, rsum, fnb)

            def post_phase(b):
                e_sb, e8, rsum, fnb = soft.pop(b)

                # wT [128, NL]: transpose of scattered e8 (or direct column
                # matmuls for the last batch, whose scatter would be exposed)
                wt_ps = pwt2.tile([P, NU * BPC], F32, tag="wt2", name="wt_ps")
                if e8 is not None:
                    nc.tensor.matmul(
                        wt_ps[:, :NL],
                        e8[:],
                        ident8[:],
                        is_transpose=True,
                        start=True,
                        stop=True,
                    )
                else:
                    for lt in range(NL):
                        nc.tensor.matmul(
                            wt_ps[:, lt : lt + 1],
                            e_sb[:, lt * P : (lt + 1) * P],
                            ones_sb[:],
                            start=(lt == 0),
                            stop=(lt == NL - 1),
                        )
                wt_sb = small.tile([P, NL], F32R, tag="wtsb", name="wt_sb")
                nc.vector.tensor_copy(wt_sb[:], wt_ps[:, :NL])

                # attention weights output
                aw_sb = outp.tile([1, L], F32, tag="aw", name="aw_sb")
                nc.vector.tensor_scalar_mul(aw_sb[:], e_sb[:], rsum[:])
                nc.sync.dma_start(attn_out[b : b + 1, :], aw_sb[:])

                # context: ctx[d] = sum_l w[l] * fN[l, d]
                ctx_sb = outp.tile([1, D], F32, tag="ctxsb", name="ctx_sb")
                for dh in range(2):
                    ctx_ps = pctx.tile([1, 512], F32, tag="ctx", name="ctx_ps")
                    for lt in range(NL):
                        nc.tensor.matmul(
                            ctx_ps[:],
                            wt_sb[:, lt : lt + 1],
                            fnb[:, lt, dh * 512 : (dh + 1) * 512],
                            start=(lt == 0),
                            stop=(lt == NL - 1),
                        )
                    nc.vector.tensor_scalar_mul(
                        ctx_sb[:, dh * 512 : (dh + 1) * 512], ctx_ps[:], rsum[:]
                    )
                nc.sync.dma_start(ctx_out[b : b + 1, :], ctx_sb[:])

            for b in range(BPC):
                proj_phase(b)
                if b > 0:
                    post_phase(b - 1)
            post_phase(BPC - 1)

    nc.finalize()
    return nc


def _get_program():
    if "nc" not in _PROGRAM_CACHE:
        _PROGRAM_CACHE["nc"] = build_program()
    return _PROGRAM_CACHE["nc"]


def _prep_in_maps(features, hidden, W1, b1, W2, b2, V, bV):
    features = np.asarray(features, dtype=np.float32)
    hidden = np.asarray(hidden, dtype=np.float32)
    W1 = np.asarray(W1, dtype=np.float32)
    b1 = np.asarray(b1, dtype=np.float32)
    W2 = np.asarray(W2, dtype=np.float32)
    b2 = np.asarray(b2, dtype=np.float32)
    V = np.asarray(V, dtype=np.float32)

    # W2aug: rows of W2, then a row of (b1 + b2), zero-padded to KH*128 rows,
    # pre-arranged to [chunk c, partition p, k-in-chunk, u].
    W2aug = np.zeros((KH * P, UNITS), dtype=np.float32)
    W2aug[:H] = W2
    W2aug[H] = b1 + b2
    W2aug = np.ascontiguousarray(
        W2aug.reshape(3, 3, P, UNITS).transpose(0, 2, 1, 3)
    )
    W1p = np.ascontiguousarray(
        W1.reshape(KD, P, 2, 512).transpose(2, 1, 0, 3)
    )
    Vp = np.ascontiguousarray(V.reshape(NU, P).T)

    in_maps = []
    for c in range(NCORES):
        sl = slice(c * BPC, (c + 1) * BPC)
        feats = features[sl]
        fT = np.ascontiguousarray(
            feats.transpose(0, 2, 1)
            .reshape(BPC, KD, P, 2, 512)
            .transpose(0, 3, 2, 1, 4)
        )
        fN = np.ascontiguousarray(
            feats.reshape(BPC, NL, P, D).transpose(0, 2, 1, 3)
        )
        hidT = np.zeros((KH * P, BPC), dtype=np.float32)
        hidT[:H] = hidden[sl].T
        hidT[H] = 1.0
        hidT = np.ascontiguousarray(hidT.reshape(KH, P, BPC).transpose(1, 0, 2))
        in_maps.append(
            {
                "fT": fT,
                "fN": fN,
                "W1": W1p,
                "W2aug": W2aug,
                "hidT": hidT,
                "Vp": Vp,
            }
        )
    return in_maps


def kernel(features, hidden, W1, b1, W2, b2, V, bV, _trace=False):
    nc = _get_program()
    in_maps = _prep_in_maps(features, hidden, W1, b1, W2, b2, V, bV)
    res = run_bass_kernel_spmd(
        nc, in_maps, core_ids=list(range(NCORES)), trace=_trace
    )
    ctx = np.concatenate([res.results[c]["ctx"] for c in range(NCORES)], axis=0)
    attn = np.concatenate([res.results[c]["attn"] for c in range(NCORES)], axis=0)
    out = (
        np.ascontiguousarray(ctx, dtype=np.float32),
        np.ascontiguousarray(attn.reshape(B, L, 1), dtype=np.float32),
    )
    if _trace:
        return out, res
    return out
